# revision 19
# baseline (speedup 1.0000x reference)
"""Trainium2 Bass kernel for CrossAttentionConditionInjection.

Math note: in the reference, K and V are projections of a single per-batch
condition vector broadcast identically across all S key positions.  The
attention scores are therefore constant along the softmax axis, softmax is
exactly uniform (1/S each), and the attention output is the mean of S
identical V rows, i.e. V itself.  The whole module collapses exactly to

    out[b, s, :] = (condition[b] @ Wv.T + bv) @ Wo.T + bo      (for every s)

independent of hidden_states / Wq / bq / Wk / bk.  (S = 1024 is a power of
two, so even the fp32 softmax-average path is bit-exact against this.)

Device strategy (8 NeuronCores on one trn2 chip, SPMD, two small NEFFs —
a collective-based single NEFF was measured slower: any collective costs
~80us wall in this runtime, while a whole no-collective NEFF is ~12us):

  Launch A: Wv.T column-sharded 8x.  Core i computes
            vT[256i:256(i+1), :] = (condition @ Wv.T + bv).T[shard]
            and returns the (256, 4) shard.  Host concatenates to the
            full (2048, 4) vT (layout only).
  Launch B: Wo.T column-sharded 8x.  Core i computes
            r[:, shard] = vT.T @ Wo.T[:, shard], folds bo + the
            broadcast over sequence positions into one selector matmul
            per batch entry, and writes its (4, 1024, 256) output
            slice.  Host concatenates along channels (layout only).
"""

import numpy as np

import concourse.bass as bass
import concourse.mybir as mybir
import concourse.tile as tile
from concourse import bacc
from concourse.bass_utils import run_bass_kernel_spmd
from concourse.masks import make_identity

B = 4
S = 1024
D = 2048
N_CORES = 8
JC = D // N_CORES  # 256 channels per core (v-shard in A, out-shard in B)
P = 128
KT = D // P  # 16 k-chunks
FP = mybir.dt.float32


def build_nc_a():
    nc = bacc.Bacc(
        "TRN2",
        target_bir_lowering=False,
        debug=False,
        enable_asserts=False,
        num_devices=N_CORES,
    )

    ct_d = nc.dram_tensor("ct", [D, B], FP, kind="ExternalInput").ap()
    wv_d = nc.dram_tensor("wv_s", [D, JC], FP, kind="ExternalInput").ap()
    bv_d = nc.dram_tensor("bv_s", [JC], FP, kind="ExternalInput").ap()
    vt_d = nc.dram_tensor("vt_s", [JC, B], FP, kind="ExternalOutput").ap()

    with tile.TileContext(nc) as tc:
        with (
            tc.tile_pool(name="work", bufs=1) as work,
            tc.tile_pool(name="pv", bufs=1, space="PSUM") as pv_pool,
            tc.tile_pool(name="pt", bufs=2, space="PSUM") as pt_pool,
        ):
            wv_sb = work.tile([P, KT, JC], FP)
            ct_sb = work.tile([P, KT, B], FP)
            bv_sb = work.tile([P, JC // P], FP)
            vl_sb = work.tile([B, JC], FP)
            vtl_sb = work.tile([P, JC // P, B], FP)
            id4_sb = work.tile([B, B], FP)
            make_identity(nc, id4_sb[:, :])

            # PE warmup: ~4us of junk matmuls while the input DMAs stream,
            # so the HAM clock gate is at 2.4GHz when the real matmuls start
            wup_sb = work.tile([P, P], FP)
            nc.vector.memset(wup_sb[:, :], 0.0)
            for w in range(8):
                pw = pt_pool.tile([P, P], FP)
                nc.tensor.matmul(
                    pw[:, :], wup_sb[:, :], wup_sb[:, :], start=True, stop=True
                )

            nc.sync.dma_start(ct_sb[:, :, :], ct_d.rearrange("(t p) b -> p t b", p=P))
            for t in range(KT):
                nc.sync.dma_start(wv_sb[:, t, :], wv_d[t * P : (t + 1) * P, :])
            nc.sync.dma_start(bv_sb[:, :], bv_d.rearrange("(g p) -> p g", p=P))

            # vl[b, j] = sum_k cT[k, b] * WvT_shard[k, j]
            pv = pv_pool.tile([B, JC], FP)
            for t in range(KT):
                nc.tensor.matmul(
                    pv[:, :],
                    ct_sb[:, t, :],
                    wv_sb[:, t, :],
                    start=(t == 0),
                    stop=(t == KT - 1),
                )
            nc.vector.tensor_copy(vl_sb[:, :], pv[:, :])

            # transpose + bv, then store the vT shard
            for g in range(JC // P):
                pt = pt_pool.tile([P, B], FP)
                nc.tensor.transpose(
                    pt[:, :], vl_sb[:, g * P : (g + 1) * P], id4_sb[:, :]
                )
                nc.vector.tensor_scalar_add(
                    vtl_sb[:, g, :], pt[:, :], bv_sb[:, g : g + 1]
                )
            nc.sync.dma_start(
                vt_d.rearrange("(g p) b -> p g b", p=P), vtl_sb[:, :, :]
            )

    nc.compile()
    return nc


def build_nc_b():
    nc = bacc.Bacc(
        "TRN2",
        target_bir_lowering=False,
        debug=False,
        enable_asserts=False,
        num_devices=N_CORES,
    )

    vt_d = nc.dram_tensor("vt", [D, B], FP, kind="ExternalInput").ap()
    wo_d = nc.dram_tensor("wo_s", [D, JC], FP, kind="ExternalInput").ap()
    bo_d = nc.dram_tensor("bo_s", [1, JC], FP, kind="ExternalInput").ap()
    sel_d = nc.dram_tensor("sel", [B + 1, B * P], FP, kind="ExternalInput").ap()
    out_d = nc.dram_tensor("out", [B, S, JC], FP, kind="ExternalOutput").ap()

    with tile.TileContext(nc) as tc:
        with (
            tc.tile_pool(name="work", bufs=1) as work,
            tc.tile_pool(name="pr", bufs=2, space="PSUM") as pr_pool,
            tc.tile_pool(name="pb", bufs=2, space="PSUM") as pb_pool,
        ):
            wo_sb = work.tile([P, KT, JC], FP)
            vt_sb = work.tile([P, KT, B], FP)
            rb_sb = work.tile([B + 1, JC], FP)
            sel_sb = work.tile([B + 1, B * P], FP)
            bc_sb = work.tile([P, B, JC], FP)

            # PE warmup (see launch A)
            wup_sb = work.tile([P, P], FP)
            nc.vector.memset(wup_sb[:, :], 0.0)
            for w in range(8):
                pw = pb_pool.tile([P, JC], FP)
                nc.tensor.matmul(
                    pw[:, 0:P], wup_sb[:, :], wup_sb[:, :], start=True, stop=True
                )

            nc.sync.dma_start(vt_sb[:, :, :], vt_d.rearrange("(g p) b -> p g b", p=P))
            for g in range(KT):
                nc.sync.dma_start(wo_sb[:, g, :], wo_d[g * P : (g + 1) * P, :])
            nc.sync.dma_start(rb_sb[B : B + 1, :], bo_d[:, :])
            nc.sync.dma_start(sel_sb[:, :], sel_d[:, :])

            # r[b, j] = sum_d vT[d, b] * WoT_shard[d, j], split in column
            # halves so the first half's broadcast+stores overlap the
            # second half's matmuls
            JH = JC // 2
            for h in range(2):
                jsl = slice(h * JH, (h + 1) * JH)
                pr = pr_pool.tile([B, JH], FP)
                for g in range(KT):
                    nc.tensor.matmul(
                        pr[:, :],
                        vt_sb[:, g, :],
                        wo_sb[:, g, jsl],
                        start=(g == 0),
                        stop=(g == KT - 1),
                    )
                nc.vector.tensor_copy(rb_sb[0:B, jsl], pr[:, :])

                # broadcast (+ bo) via selector matmul, interleaved stores
                for b in range(B):
                    pb = pb_pool.tile([P, JH], FP)
                    nc.tensor.matmul(
                        pb[:, :],
                        sel_sb[:, b * P : (b + 1) * P],
                        rb_sb[:, jsl],
                        start=True,
                        stop=True,
                    )
                    nc.vector.tensor_copy(bc_sb[:, b, jsl], pb[:, :])
                    for sc in range(S // P):
                        nc.sync.dma_start(
                            out_d[b, sc * P : (sc + 1) * P, jsl],
                            bc_sb[:, b, jsl],
                        )

    nc.compile()
    return nc


def make_in_maps_a(condition, Wv, bv):
    ct = np.ascontiguousarray(np.asarray(condition, dtype=np.float32).T)
    wvT = np.asarray(Wv, dtype=np.float32).T
    bv = np.asarray(bv, dtype=np.float32)
    in_maps = []
    for i in range(N_CORES):
        sl = slice(i * JC, (i + 1) * JC)
        in_maps.append(
            {
                "ct": ct,
                "wv_s": np.ascontiguousarray(wvT[:, sl]),
                "bv_s": np.ascontiguousarray(bv[sl]),
            }
        )
    return in_maps


def make_in_maps_b(vt, Wo, bo):
    woT = np.asarray(Wo, dtype=np.float32).T
    bo = np.asarray(bo, dtype=np.float32)
    sel = np.zeros((B + 1, B * P), dtype=np.float32)
    for b in range(B):
        sel[b, b * P : (b + 1) * P] = 1.0
    sel[B, :] = 1.0
    in_maps = []
    for i in range(N_CORES):
        sl = slice(i * JC, (i + 1) * JC)
        in_maps.append(
            {
                "vt": vt,
                "wo_s": np.ascontiguousarray(woT[:, sl]),
                "bo_s": np.ascontiguousarray(bo[sl]).reshape(1, JC),
                "sel": sel,
            }
        )
    return in_maps


_NC_CACHE = None


def get_ncs():
    global _NC_CACHE
    if _NC_CACHE is None:
        _NC_CACHE = (build_nc_a(), build_nc_b())
    return _NC_CACHE


def kernel(**inputs):
    nc_a, nc_b = get_ncs()
    cores = list(range(N_CORES))

    res_a = run_bass_kernel_spmd(
        nc_a,
        make_in_maps_a(inputs["condition"], inputs["Wv"], inputs["bv"]),
        core_ids=cores,
    )
    vt = np.ascontiguousarray(
        np.concatenate([r["vt_s"] for r in res_a.results], axis=0)
    )

    res_b = run_bass_kernel_spmd(
        nc_b,
        make_in_maps_b(vt, inputs["Wo"], inputs["bo"]),
        core_ids=cores,
    )
    out = np.concatenate([r["out"] for r in res_b.results], axis=-1)
    return out


# revision 23
# speedup vs baseline: 1.0697x; 1.0697x over previous
"""Trainium2 Bass kernel for CrossAttentionConditionInjection.

Math note: in the reference, K and V are projections of a single per-batch
condition vector broadcast identically across all S key positions.  The
attention scores are therefore constant along the softmax axis, softmax is
exactly uniform (1/S each), and the attention output is the mean of S
identical V rows, i.e. V itself.  The whole module collapses exactly to

    out[b, s, :] = (condition[b] @ Wv.T + bv) @ Wo.T + bo      (for every s)

independent of hidden_states / Wq / bq / Wk / bk.  (S = 1024 is a power of
two, so even the fp32 softmax-average path is bit-exact against this.)

Device strategy (8 NeuronCores on one trn2 chip, SPMD, two small NEFFs —
a collective-based single NEFF was measured slower: any collective costs
~80us wall in this runtime, while a whole no-collective NEFF is ~12us):

  Launch A: Wv.T column-sharded 8x.  Core i computes
            vT[256i:256(i+1), :] = (condition @ Wv.T + bv).T[shard]
            and returns the (256, 4) shard.  Host concatenates to the
            full (2048, 4) vT (layout only).
  Launch B: Wo.T column-sharded 8x.  Core i computes
            r[:, shard] = vT.T @ Wo.T[:, shard], folds bo + the
            broadcast over sequence positions into one selector matmul
            per batch entry, and writes its (4, 1024, 256) output
            slice.  Host concatenates along channels (layout only).

Both launches are raw bass (manual semaphores, no TileContext): the Tile
head/tail barriers cost ~8+9us per NEFF, which dominates kernels this
small.  Set USE_RAW=False to fall back to the Tile versions.
"""

import numpy as np

import concourse.bass as bass
import concourse.mybir as mybir
import concourse.tile as tile
from concourse import bacc
from concourse.bass_utils import run_bass_kernel_spmd
from concourse.masks import make_identity

B = 4
S = 1024
D = 2048
N_CORES = 8
JC = D // N_CORES  # 256 channels per core (v-shard in A, out-shard in B)
P = 128
KT = D // P  # 16 k-chunks
FP = mybir.dt.float32

USE_RAW = True

N_WARM = 8  # junk matmuls to lift the PE HAM clock gate while DMAs stream


def _new_nc():
    return bacc.Bacc(
        "TRN2",
        target_bir_lowering=False,
        debug=False,
        enable_asserts=False,
        num_devices=N_CORES,
    )


def build_nc_a_raw():
    nc = _new_nc()
    ct_d = nc.dram_tensor("ct", [D, B], FP, kind="ExternalInput").ap()
    wv_d = nc.dram_tensor("wv_s", [D, JC], FP, kind="ExternalInput").ap()
    bv_d = nc.dram_tensor("bv_s", [P, JC // P], FP, kind="ExternalInput").ap()
    id4_d = nc.dram_tensor("id4", [B, B], FP, kind="ExternalInput").ap()
    vt_d = nc.dram_tensor("vt_s", [JC, B], FP, kind="ExternalOutput").ap()

    N_IN = 3 + KT  # ct, bv, id4, wv x16

    with (
        nc.semaphore("s_in") as s_in,
        nc.semaphore("s_wu") as s_wu,
        nc.semaphore("s_pv") as s_pv,
        nc.semaphore("s_vl") as s_vl,
        nc.semaphore("s_mm") as s_mm,
        nc.semaphore("s_vt") as s_vt,
        nc.semaphore("s_out") as s_out,
        nc.sbuf_tensor("ct_sb", [P, KT * B], FP) as ct_sb,
        nc.sbuf_tensor("wv_sb", [P, KT * JC], FP) as wv_sb,
        nc.sbuf_tensor("bv_sb", [P, JC // P], FP) as bv_sb,
        nc.sbuf_tensor("vl_sb", [B, JC], FP) as vl_sb,
        nc.sbuf_tensor("vtl_sb", [P, (JC // P) * B], FP) as vtl_sb,
        nc.sbuf_tensor("id4_sb", [B, B], FP) as id4_sb,
        nc.sbuf_tensor("wup_sb", [P, P], FP) as wup_sb,
        nc.psum_tensor("pwu", [P, 512], FP) as pwu,
        nc.psum_tensor("pv", [B, 512], FP) as pv,
        nc.psum_tensor("pt0", [P, 512], FP) as pt0,
        nc.psum_tensor("pt1", [P, 512], FP) as pt1,
        nc.Block() as block,
    ):

        @block.sync
        def _(sync):
            sync.dma_start(id4_sb[:, :], id4_d[:, :]).then_inc(s_in, 16)
            sync.dma_start(
                ct_sb[:, :].rearrange("p (t b) -> p t b", t=KT),
                ct_d.rearrange("(t p) b -> p t b", p=P),
            ).then_inc(s_in, 16)
            sync.dma_start(bv_sb[:, :], bv_d[:, :]).then_inc(s_in, 16)
            for t in range(KT):
                sync.dma_start(
                    wv_sb[:, t * JC : (t + 1) * JC], wv_d[t * P : (t + 1) * P, :]
                ).then_inc(s_in, 16)
            sync.wait_ge(s_vt, 2)
            sync.dma_start(
                vt_d.rearrange("(g p) b -> p g b", p=P),
                vtl_sb[:, :].rearrange("p (g b) -> p g b", g=JC // P),
            ).then_inc(s_out, 16)
            sync.wait_ge(s_out, 16)

        @block.vector
        def _(vector):
            vector.memset(wup_sb[:, :], 0.0).then_inc(s_wu, 1)
            vector.wait_ge(s_pv, 1)
            vector.tensor_copy(vl_sb[:, :], pv[:, 0:JC]).then_inc(s_vl, 1)
            for g in range(JC // P):
                pt = pt0 if g == 0 else pt1
                vector.wait_ge(s_mm, g + 1)
                vector.tensor_scalar_add(
                    vtl_sb[:, g * B : (g + 1) * B], pt[:, 0:B], bv_sb[:, g : g + 1]
                ).then_inc(s_vt, 1)

        @block.tensor
        def _(tensor):
            tensor.wait_ge(s_wu, 1)
            for w in range(N_WARM):
                tensor.matmul(
                    pwu[:, 0:P], wup_sb[:, :], wup_sb[:, :], start=True, stop=True
                )
            tensor.wait_ge(s_in, N_IN * 16)
            for t in range(KT):
                mm = tensor.matmul(
                    pv[:, 0:JC],
                    ct_sb[:, t * B : (t + 1) * B],
                    wv_sb[:, t * JC : (t + 1) * JC],
                    start=(t == 0),
                    stop=(t == KT - 1),
                )
            mm.then_inc(s_pv, 1)
            tensor.wait_ge(s_vl, 1)
            for g in range(JC // P):
                pt = pt0 if g == 0 else pt1
                tensor.transpose(
                    pt[:, 0:B], vl_sb[:, g * P : (g + 1) * P], id4_sb[:, :]
                ).then_inc(s_mm, 1)

    nc.compile()
    return nc


def build_nc_b_raw():
    nc = _new_nc()
    vt_d = nc.dram_tensor("vt", [D, B], FP, kind="ExternalInput").ap()
    wo_d = nc.dram_tensor("wo_s", [D, JC], FP, kind="ExternalInput").ap()
    bo_d = nc.dram_tensor("bo_s", [1, JC], FP, kind="ExternalInput").ap()
    sel_d = nc.dram_tensor("sel", [B + 1, B * P], FP, kind="ExternalInput").ap()
    out_d = nc.dram_tensor("out", [B, S, JC], FP, kind="ExternalOutput").ap()

    N_IN = 3 + KT  # vt, bo, sel, wo x16

    with (
        nc.semaphore("s_in") as s_in,
        nc.semaphore("s_wu") as s_wu,
        nc.semaphore("s_r") as s_r,
        nc.semaphore("s_rb") as s_rb,
        nc.semaphore("s_bct") as s_bct,
        nc.semaphore("s_bc") as s_bc,
        nc.semaphore("s_out") as s_out,
        nc.sbuf_tensor("vt_sb", [P, KT * B], FP) as vt_sb,
        nc.sbuf_tensor("wo_sb", [P, KT * JC], FP) as wo_sb,
        nc.sbuf_tensor("rb_sb", [B + 1, JC], FP) as rb_sb,
        nc.sbuf_tensor("sel_sb", [B + 1, B * P], FP) as sel_sb,
        nc.sbuf_tensor("bc_sb", [P, B * JC], FP) as bc_sb,
        nc.sbuf_tensor("wup_sb", [P, P], FP) as wup_sb,
        nc.psum_tensor("pwu", [P, 512], FP) as pwu,
        nc.psum_tensor("pr", [B, 512], FP) as pr,
        nc.psum_tensor("pb0", [P, 512], FP) as pb0,
        nc.psum_tensor("pb1", [P, 512], FP) as pb1,
        nc.Block() as block,
    ):

        @block.sync
        def _(sync):
            sync.dma_start(
                vt_sb[:, :].rearrange("p (g b) -> p g b", g=KT),
                vt_d.rearrange("(g p) b -> p g b", p=P),
            ).then_inc(s_in, 16)
            sync.dma_start(rb_sb[B : B + 1, :], bo_d[:, :]).then_inc(s_in, 16)
            sync.dma_start(sel_sb[:, :], sel_d[:, :]).then_inc(s_in, 16)
            for g in range(KT):
                sync.dma_start(
                    wo_sb[:, g * JC : (g + 1) * JC], wo_d[g * P : (g + 1) * P, :]
                ).then_inc(s_in, 16)
            for b in range(B):
                sync.wait_ge(s_bc, b + 1)
                for sc in range(S // P):
                    sync.dma_start(
                        out_d[b, sc * P : (sc + 1) * P, :],
                        bc_sb[:, b * JC : (b + 1) * JC],
                    ).then_inc(s_out, 16)
            sync.wait_ge(s_out, B * (S // P) * 16)

        @block.vector
        def _(vector):
            vector.memset(wup_sb[:, :], 0.0).then_inc(s_wu, 1)
            vector.wait_ge(s_r, 1)
            vector.tensor_copy(rb_sb[0:B, :], pr[:, 0:JC]).then_inc(s_rb, 1)
            for b in range(B):
                pb = pb0 if b % 2 == 0 else pb1
                vector.wait_ge(s_bct, b + 1)
                vector.tensor_copy(
                    bc_sb[:, b * JC : (b + 1) * JC], pb[:, 0:JC]
                ).then_inc(s_bc, 1)

        @block.tensor
        def _(tensor):
            tensor.wait_ge(s_wu, 1)
            for w in range(N_WARM):
                tensor.matmul(
                    pwu[:, 0:P], wup_sb[:, :], wup_sb[:, :], start=True, stop=True
                )
            tensor.wait_ge(s_in, N_IN * 16)
            for g in range(KT):
                mm = tensor.matmul(
                    pr[:, 0:JC],
                    vt_sb[:, g * B : (g + 1) * B],
                    wo_sb[:, g * JC : (g + 1) * JC],
                    start=(g == 0),
                    stop=(g == KT - 1),
                )
            mm.then_inc(s_r, 1)
            tensor.wait_ge(s_rb, 1)
            for b in range(B):
                pb = pb0 if b % 2 == 0 else pb1
                if b >= 2:
                    tensor.wait_ge(s_bc, b - 1)
                tensor.matmul(
                    pb[:, 0:JC],
                    sel_sb[:, b * P : (b + 1) * P],
                    rb_sb[:, :],
                    start=True,
                    stop=True,
                ).then_inc(s_bct, 1)

    nc.compile()
    return nc


def build_nc_a_tile():
    nc = _new_nc()
    ct_d = nc.dram_tensor("ct", [D, B], FP, kind="ExternalInput").ap()
    wv_d = nc.dram_tensor("wv_s", [D, JC], FP, kind="ExternalInput").ap()
    bv_d = nc.dram_tensor("bv_s", [P, JC // P], FP, kind="ExternalInput").ap()
    id4_d = nc.dram_tensor("id4", [B, B], FP, kind="ExternalInput").ap()
    vt_d = nc.dram_tensor("vt_s", [JC, B], FP, kind="ExternalOutput").ap()

    with tile.TileContext(nc) as tc:
        with (
            tc.tile_pool(name="work", bufs=1) as work,
            tc.tile_pool(name="pv", bufs=1, space="PSUM") as pv_pool,
            tc.tile_pool(name="pt", bufs=2, space="PSUM") as pt_pool,
        ):
            wv_sb = work.tile([P, KT, JC], FP)
            ct_sb = work.tile([P, KT, B], FP)
            bv_sb = work.tile([P, JC // P], FP)
            vl_sb = work.tile([B, JC], FP)
            vtl_sb = work.tile([P, JC // P, B], FP)
            id4_sb = work.tile([B, B], FP)
            nc.sync.dma_start(id4_sb[:, :], id4_d[:, :])

            nc.sync.dma_start(ct_sb[:, :, :], ct_d.rearrange("(t p) b -> p t b", p=P))
            for t in range(KT):
                nc.sync.dma_start(wv_sb[:, t, :], wv_d[t * P : (t + 1) * P, :])
            nc.sync.dma_start(bv_sb[:, :], bv_d[:, :])

            pv = pv_pool.tile([B, JC], FP)
            for t in range(KT):
                nc.tensor.matmul(
                    pv[:, :],
                    ct_sb[:, t, :],
                    wv_sb[:, t, :],
                    start=(t == 0),
                    stop=(t == KT - 1),
                )
            nc.vector.tensor_copy(vl_sb[:, :], pv[:, :])

            for g in range(JC // P):
                pt = pt_pool.tile([P, B], FP)
                nc.tensor.transpose(
                    pt[:, :], vl_sb[:, g * P : (g + 1) * P], id4_sb[:, :]
                )
                nc.vector.tensor_scalar_add(
                    vtl_sb[:, g, :], pt[:, :], bv_sb[:, g : g + 1]
                )
            nc.sync.dma_start(
                vt_d.rearrange("(g p) b -> p g b", p=P), vtl_sb[:, :, :]
            )

    nc.compile()
    return nc


def build_nc_b_tile():
    nc = _new_nc()
    vt_d = nc.dram_tensor("vt", [D, B], FP, kind="ExternalInput").ap()
    wo_d = nc.dram_tensor("wo_s", [D, JC], FP, kind="ExternalInput").ap()
    bo_d = nc.dram_tensor("bo_s", [1, JC], FP, kind="ExternalInput").ap()
    sel_d = nc.dram_tensor("sel", [B + 1, B * P], FP, kind="ExternalInput").ap()
    out_d = nc.dram_tensor("out", [B, S, JC], FP, kind="ExternalOutput").ap()

    with tile.TileContext(nc) as tc:
        with (
            tc.tile_pool(name="work", bufs=1) as work,
            tc.tile_pool(name="pr", bufs=1, space="PSUM") as pr_pool,
            tc.tile_pool(name="pb", bufs=2, space="PSUM") as pb_pool,
        ):
            wo_sb = work.tile([P, KT, JC], FP)
            vt_sb = work.tile([P, KT, B], FP)
            rb_sb = work.tile([B + 1, JC], FP)
            sel_sb = work.tile([B + 1, B * P], FP)
            bc_sb = work.tile([P, B, JC], FP)

            nc.sync.dma_start(vt_sb[:, :, :], vt_d.rearrange("(g p) b -> p g b", p=P))
            for g in range(KT):
                nc.sync.dma_start(wo_sb[:, g, :], wo_d[g * P : (g + 1) * P, :])
            nc.sync.dma_start(rb_sb[B : B + 1, :], bo_d[:, :])
            nc.sync.dma_start(sel_sb[:, :], sel_d[:, :])

            pr = pr_pool.tile([B, JC], FP)
            for g in range(KT):
                nc.tensor.matmul(
                    pr[:, :],
                    vt_sb[:, g, :],
                    wo_sb[:, g, :],
                    start=(g == 0),
                    stop=(g == KT - 1),
                )
            nc.vector.tensor_copy(rb_sb[0:B, :], pr[:, :])

            for b in range(B):
                pb = pb_pool.tile([P, JC], FP)
                nc.tensor.matmul(
                    pb[:, :],
                    sel_sb[:, b * P : (b + 1) * P],
                    rb_sb[:, :],
                    start=True,
                    stop=True,
                )
                nc.vector.tensor_copy(bc_sb[:, b, :], pb[:, :])
                for sc in range(S // P):
                    nc.sync.dma_start(
                        out_d[b, sc * P : (sc + 1) * P, :], bc_sb[:, b, :]
                    )

    nc.compile()
    return nc


def build_nc_a():
    return build_nc_a_raw() if USE_RAW else build_nc_a_tile()


def build_nc_b():
    return build_nc_b_raw() if USE_RAW else build_nc_b_tile()


def make_in_maps_a(condition, Wv, bv):
    ct = np.ascontiguousarray(np.asarray(condition, dtype=np.float32).T)
    wvT = np.asarray(Wv, dtype=np.float32).T
    bv = np.asarray(bv, dtype=np.float32)
    id4 = np.eye(B, dtype=np.float32)
    in_maps = []
    for i in range(N_CORES):
        sl = slice(i * JC, (i + 1) * JC)
        in_maps.append(
            {
                "ct": ct,
                "wv_s": np.ascontiguousarray(wvT[:, sl]),
                "bv_s": np.ascontiguousarray(bv[sl].reshape(JC // P, P).T),
                "id4": id4,
            }
        )
    return in_maps


def make_in_maps_b(vt, Wo, bo):
    woT = np.asarray(Wo, dtype=np.float32).T
    bo = np.asarray(bo, dtype=np.float32)
    sel = np.zeros((B + 1, B * P), dtype=np.float32)
    for b in range(B):
        sel[b, b * P : (b + 1) * P] = 1.0
    sel[B, :] = 1.0
    in_maps = []
    for i in range(N_CORES):
        sl = slice(i * JC, (i + 1) * JC)
        in_maps.append(
            {
                "vt": vt,
                "wo_s": np.ascontiguousarray(woT[:, sl]),
                "bo_s": np.ascontiguousarray(bo[sl]).reshape(1, JC),
                "sel": sel,
            }
        )
    return in_maps


_NC_CACHE = None


def get_ncs():
    global _NC_CACHE
    if _NC_CACHE is None:
        _NC_CACHE = (build_nc_a(), build_nc_b())
    return _NC_CACHE


def kernel(**inputs):
    nc_a, nc_b = get_ncs()
    cores = list(range(N_CORES))

    res_a = run_bass_kernel_spmd(
        nc_a,
        make_in_maps_a(inputs["condition"], inputs["Wv"], inputs["bv"]),
        core_ids=cores,
    )
    vt = np.ascontiguousarray(
        np.concatenate([r["vt_s"] for r in res_a.results], axis=0)
    )

    res_b = run_bass_kernel_spmd(
        nc_b,
        make_in_maps_b(vt, inputs["Wo"], inputs["bo"]),
        core_ids=cores,
    )
    out = np.concatenate([r["out"] for r in res_b.results], axis=-1)
    return out


# revision 24
# speedup vs baseline: 1.1417x; 1.0673x over previous
"""Trainium2 Bass kernel for CrossAttentionConditionInjection.

Math note: in the reference, K and V are projections of a single per-batch
condition vector broadcast identically across all S key positions.  The
attention scores are therefore constant along the softmax axis, softmax is
exactly uniform (1/S each), and the attention output is the mean of S
identical V rows, i.e. V itself.  The whole module collapses exactly to

    out[b, s, :] = (condition[b] @ Wv.T + bv) @ Wo.T + bo      (for every s)

independent of hidden_states / Wq / bq / Wk / bk.  (S = 1024 is a power of
two, so even the fp32 softmax-average path is bit-exact against this.)

Device strategy (8 NeuronCores on one trn2 chip, SPMD, two small NEFFs —
a collective-based single NEFF was measured slower: any collective costs
~80us wall in this runtime, while a whole no-collective NEFF is ~12us):

  Launch A: Wv.T column-sharded 8x.  Core i computes
            vT[256i:256(i+1), :] = (condition @ Wv.T + bv).T[shard]
            and returns the (256, 4) shard.  Host concatenates to the
            full (2048, 4) vT (layout only).
  Launch B: Wo.T column-sharded 8x.  Core i computes
            r[:, shard] = vT.T @ Wo.T[:, shard], folds bo + the
            broadcast over sequence positions into one selector matmul
            per batch entry, and writes its (4, 1024, 256) output
            slice.  Host concatenates along channels (layout only).

Both launches are raw bass (manual semaphores, no TileContext): the Tile
head/tail barriers cost ~8+9us per NEFF, which dominates kernels this
small.  Set USE_RAW=False to fall back to the Tile versions.
"""

import numpy as np

import concourse.bass as bass
import concourse.mybir as mybir
import concourse.tile as tile
from concourse import bacc
from concourse.bass_utils import run_bass_kernel_spmd
from concourse.masks import make_identity

B = 4
S = 1024
D = 2048
N_CORES = 8
JC = D // N_CORES  # 256 channels per core (v-shard in A, out-shard in B)
P = 128
KT = D // P  # 16 k-chunks
FP = mybir.dt.float32

USE_RAW = True

N_WARM = 8  # junk matmuls to lift the PE HAM clock gate while DMAs stream


def _new_nc():
    return bacc.Bacc(
        "TRN2",
        target_bir_lowering=False,
        debug=False,
        enable_asserts=False,
        num_devices=N_CORES,
    )


def build_nc_a_raw():
    nc = _new_nc()
    ct_d = nc.dram_tensor("ct", [D, B], FP, kind="ExternalInput").ap()
    wv_d = nc.dram_tensor("wv_s", [D, JC], FP, kind="ExternalInput").ap()
    bv_d = nc.dram_tensor("bv_s", [P, JC // P], FP, kind="ExternalInput").ap()
    id4_d = nc.dram_tensor("id4", [B, B], FP, kind="ExternalInput").ap()
    vt_d = nc.dram_tensor("vt_s", [JC, B], FP, kind="ExternalOutput").ap()

    N_IN = 3 + KT  # ct, bv, id4, wv x16

    with (
        nc.semaphore("s_in") as s_in,
        nc.semaphore("s_h0") as s_h0,
        nc.semaphore("s_h1") as s_h1,
        nc.semaphore("s_wu") as s_wu,
        nc.semaphore("s_pv") as s_pv,
        nc.semaphore("s_vl") as s_vl,
        nc.semaphore("s_mm") as s_mm,
        nc.semaphore("s_vt") as s_vt,
        nc.semaphore("s_out") as s_out,
        nc.sbuf_tensor("ct_sb", [P, KT * B], FP) as ct_sb,
        nc.sbuf_tensor("wv_sb", [P, KT * JC], FP) as wv_sb,
        nc.sbuf_tensor("bv_sb", [P, JC // P], FP) as bv_sb,
        nc.sbuf_tensor("vl_sb", [B, JC], FP) as vl_sb,
        nc.sbuf_tensor("vtl_sb", [P, (JC // P) * B], FP) as vtl_sb,
        nc.sbuf_tensor("id4_sb", [B, B], FP) as id4_sb,
        nc.sbuf_tensor("wup_sb", [P, P], FP) as wup_sb,
        nc.psum_tensor("pwu", [P, 512], FP) as pwu,
        nc.psum_tensor("pv", [B, 512], FP) as pv,
        nc.psum_tensor("pt0", [P, 512], FP) as pt0,
        nc.psum_tensor("pt1", [P, 512], FP) as pt1,
        nc.Block() as block,
    ):

        @block.sync
        def _(sync):
            sync.dma_start(id4_sb[:, :], id4_d[:, :]).then_inc(s_in, 16)
            sync.dma_start(
                ct_sb[:, :].rearrange("p (t b) -> p t b", t=KT),
                ct_d.rearrange("(t p) b -> p t b", p=P),
            ).then_inc(s_in, 16)
            sync.dma_start(bv_sb[:, :], bv_d[:, :]).then_inc(s_in, 16)
            for t in range(KT):
                sync.dma_start(
                    wv_sb[:, t * JC : (t + 1) * JC], wv_d[t * P : (t + 1) * P, :]
                ).then_inc(s_h0 if t < KT // 2 else s_h1, 16)
            sync.wait_ge(s_vt, 2)
            sync.dma_start(
                vt_d.rearrange("(g p) b -> p g b", p=P),
                vtl_sb[:, :].rearrange("p (g b) -> p g b", g=JC // P),
            ).then_inc(s_out, 16)
            sync.wait_ge(s_out, 16)

        @block.vector
        def _(vector):
            vector.memset(wup_sb[:, :], 0.0).then_inc(s_wu, 1)
            vector.wait_ge(s_pv, 1)
            vector.tensor_copy(vl_sb[:, :], pv[:, 0:JC]).then_inc(s_vl, 1)
            for g in range(JC // P):
                pt = pt0 if g == 0 else pt1
                vector.wait_ge(s_mm, g + 1)
                vector.tensor_scalar_add(
                    vtl_sb[:, g * B : (g + 1) * B], pt[:, 0:B], bv_sb[:, g : g + 1]
                ).then_inc(s_vt, 1)

        @block.tensor
        def _(tensor):
            tensor.wait_ge(s_wu, 1)
            for w in range(N_WARM):
                tensor.matmul(
                    pwu[:, 0:P], wup_sb[:, :], wup_sb[:, :], start=True, stop=True
                )
            tensor.wait_ge(s_in, 3 * 16)
            tensor.wait_ge(s_h0, (KT // 2) * 16)
            for t in range(KT):
                if t == KT // 2:
                    tensor.wait_ge(s_h1, (KT // 2) * 16)
                mm = tensor.matmul(
                    pv[:, 0:JC],
                    ct_sb[:, t * B : (t + 1) * B],
                    wv_sb[:, t * JC : (t + 1) * JC],
                    start=(t == 0),
                    stop=(t == KT - 1),
                )
            mm.then_inc(s_pv, 1)
            tensor.wait_ge(s_vl, 1)
            for g in range(JC // P):
                pt = pt0 if g == 0 else pt1
                tensor.transpose(
                    pt[:, 0:B], vl_sb[:, g * P : (g + 1) * P], id4_sb[:, :]
                ).then_inc(s_mm, 1)

    nc.compile()
    return nc


def build_nc_b_raw():
    nc = _new_nc()
    vt_d = nc.dram_tensor("vt", [D, B], FP, kind="ExternalInput").ap()
    wo_d = nc.dram_tensor("wo_s", [D, JC], FP, kind="ExternalInput").ap()
    bo_d = nc.dram_tensor("bo_s", [1, JC], FP, kind="ExternalInput").ap()
    sel_d = nc.dram_tensor("sel", [B + 1, B * P], FP, kind="ExternalInput").ap()
    out_d = nc.dram_tensor("out", [B, S, JC], FP, kind="ExternalOutput").ap()

    N_IN = 3 + KT  # vt, bo, sel, wo x16

    with (
        nc.semaphore("s_in") as s_in,
        nc.semaphore("s_h0") as s_h0,
        nc.semaphore("s_h1") as s_h1,
        nc.semaphore("s_wu") as s_wu,
        nc.semaphore("s_r") as s_r,
        nc.semaphore("s_rb") as s_rb,
        nc.semaphore("s_bct") as s_bct,
        nc.semaphore("s_bc") as s_bc,
        nc.semaphore("s_out") as s_out,
        nc.sbuf_tensor("vt_sb", [P, KT * B], FP) as vt_sb,
        nc.sbuf_tensor("wo_sb", [P, KT * JC], FP) as wo_sb,
        nc.sbuf_tensor("rb_sb", [B + 1, JC], FP) as rb_sb,
        nc.sbuf_tensor("sel_sb", [B + 1, B * P], FP) as sel_sb,
        nc.sbuf_tensor("bc_sb", [P, B * JC], FP) as bc_sb,
        nc.sbuf_tensor("wup_sb", [P, P], FP) as wup_sb,
        nc.psum_tensor("pwu", [P, 512], FP) as pwu,
        nc.psum_tensor("pr", [B, 512], FP) as pr,
        nc.psum_tensor("pb0", [P, 512], FP) as pb0,
        nc.psum_tensor("pb1", [P, 512], FP) as pb1,
        nc.Block() as block,
    ):

        @block.sync
        def _(sync):
            sync.dma_start(
                vt_sb[:, :].rearrange("p (g b) -> p g b", g=KT),
                vt_d.rearrange("(g p) b -> p g b", p=P),
            ).then_inc(s_in, 16)
            sync.dma_start(rb_sb[B : B + 1, :], bo_d[:, :]).then_inc(s_in, 16)
            sync.dma_start(sel_sb[:, :], sel_d[:, :]).then_inc(s_in, 16)
            for g in range(KT):
                sync.dma_start(
                    wo_sb[:, g * JC : (g + 1) * JC], wo_d[g * P : (g + 1) * P, :]
                ).then_inc(s_h0 if g < KT // 2 else s_h1, 16)
            for b in range(B):
                sync.wait_ge(s_bc, b + 1)
                for sc in range(S // P):
                    sync.dma_start(
                        out_d[b, sc * P : (sc + 1) * P, :],
                        bc_sb[:, b * JC : (b + 1) * JC],
                    ).then_inc(s_out, 16)
            sync.wait_ge(s_out, B * (S // P) * 16)

        @block.vector
        def _(vector):
            vector.memset(wup_sb[:, :], 0.0).then_inc(s_wu, 1)
            vector.wait_ge(s_r, 1)
            vector.tensor_copy(rb_sb[0:B, :], pr[:, 0:JC]).then_inc(s_rb, 1)
            for b in range(B):
                pb = pb0 if b % 2 == 0 else pb1
                vector.wait_ge(s_bct, b + 1)
                vector.tensor_copy(
                    bc_sb[:, b * JC : (b + 1) * JC], pb[:, 0:JC]
                ).then_inc(s_bc, 1)

        @block.tensor
        def _(tensor):
            tensor.wait_ge(s_wu, 1)
            for w in range(N_WARM):
                tensor.matmul(
                    pwu[:, 0:P], wup_sb[:, :], wup_sb[:, :], start=True, stop=True
                )
            tensor.wait_ge(s_in, 3 * 16)
            tensor.wait_ge(s_h0, (KT // 2) * 16)
            for g in range(KT):
                if g == KT // 2:
                    tensor.wait_ge(s_h1, (KT // 2) * 16)
                mm = tensor.matmul(
                    pr[:, 0:JC],
                    vt_sb[:, g * B : (g + 1) * B],
                    wo_sb[:, g * JC : (g + 1) * JC],
                    start=(g == 0),
                    stop=(g == KT - 1),
                )
            mm.then_inc(s_r, 1)
            tensor.wait_ge(s_rb, 1)
            for b in range(B):
                pb = pb0 if b % 2 == 0 else pb1
                if b >= 2:
                    tensor.wait_ge(s_bc, b - 1)
                tensor.matmul(
                    pb[:, 0:JC],
                    sel_sb[:, b * P : (b + 1) * P],
                    rb_sb[:, :],
                    start=True,
                    stop=True,
                ).then_inc(s_bct, 1)

    nc.compile()
    return nc


def build_nc_a_tile():
    nc = _new_nc()
    ct_d = nc.dram_tensor("ct", [D, B], FP, kind="ExternalInput").ap()
    wv_d = nc.dram_tensor("wv_s", [D, JC], FP, kind="ExternalInput").ap()
    bv_d = nc.dram_tensor("bv_s", [P, JC // P], FP, kind="ExternalInput").ap()
    id4_d = nc.dram_tensor("id4", [B, B], FP, kind="ExternalInput").ap()
    vt_d = nc.dram_tensor("vt_s", [JC, B], FP, kind="ExternalOutput").ap()

    with tile.TileContext(nc) as tc:
        with (
            tc.tile_pool(name="work", bufs=1) as work,
            tc.tile_pool(name="pv", bufs=1, space="PSUM") as pv_pool,
            tc.tile_pool(name="pt", bufs=2, space="PSUM") as pt_pool,
        ):
            wv_sb = work.tile([P, KT, JC], FP)
            ct_sb = work.tile([P, KT, B], FP)
            bv_sb = work.tile([P, JC // P], FP)
            vl_sb = work.tile([B, JC], FP)
            vtl_sb = work.tile([P, JC // P, B], FP)
            id4_sb = work.tile([B, B], FP)
            nc.sync.dma_start(id4_sb[:, :], id4_d[:, :])

            nc.sync.dma_start(ct_sb[:, :, :], ct_d.rearrange("(t p) b -> p t b", p=P))
            for t in range(KT):
                nc.sync.dma_start(wv_sb[:, t, :], wv_d[t * P : (t + 1) * P, :])
            nc.sync.dma_start(bv_sb[:, :], bv_d[:, :])

            pv = pv_pool.tile([B, JC], FP)
            for t in range(KT):
                nc.tensor.matmul(
                    pv[:, :],
                    ct_sb[:, t, :],
                    wv_sb[:, t, :],
                    start=(t == 0),
                    stop=(t == KT - 1),
                )
            nc.vector.tensor_copy(vl_sb[:, :], pv[:, :])

            for g in range(JC // P):
                pt = pt_pool.tile([P, B], FP)
                nc.tensor.transpose(
                    pt[:, :], vl_sb[:, g * P : (g + 1) * P], id4_sb[:, :]
                )
                nc.vector.tensor_scalar_add(
                    vtl_sb[:, g, :], pt[:, :], bv_sb[:, g : g + 1]
                )
            nc.sync.dma_start(
                vt_d.rearrange("(g p) b -> p g b", p=P), vtl_sb[:, :, :]
            )

    nc.compile()
    return nc


def build_nc_b_tile():
    nc = _new_nc()
    vt_d = nc.dram_tensor("vt", [D, B], FP, kind="ExternalInput").ap()
    wo_d = nc.dram_tensor("wo_s", [D, JC], FP, kind="ExternalInput").ap()
    bo_d = nc.dram_tensor("bo_s", [1, JC], FP, kind="ExternalInput").ap()
    sel_d = nc.dram_tensor("sel", [B + 1, B * P], FP, kind="ExternalInput").ap()
    out_d = nc.dram_tensor("out", [B, S, JC], FP, kind="ExternalOutput").ap()

    with tile.TileContext(nc) as tc:
        with (
            tc.tile_pool(name="work", bufs=1) as work,
            tc.tile_pool(name="pr", bufs=1, space="PSUM") as pr_pool,
            tc.tile_pool(name="pb", bufs=2, space="PSUM") as pb_pool,
        ):
            wo_sb = work.tile([P, KT, JC], FP)
            vt_sb = work.tile([P, KT, B], FP)
            rb_sb = work.tile([B + 1, JC], FP)
            sel_sb = work.tile([B + 1, B * P], FP)
            bc_sb = work.tile([P, B, JC], FP)

            nc.sync.dma_start(vt_sb[:, :, :], vt_d.rearrange("(g p) b -> p g b", p=P))
            for g in range(KT):
                nc.sync.dma_start(wo_sb[:, g, :], wo_d[g * P : (g + 1) * P, :])
            nc.sync.dma_start(rb_sb[B : B + 1, :], bo_d[:, :])
            nc.sync.dma_start(sel_sb[:, :], sel_d[:, :])

            pr = pr_pool.tile([B, JC], FP)
            for g in range(KT):
                nc.tensor.matmul(
                    pr[:, :],
                    vt_sb[:, g, :],
                    wo_sb[:, g, :],
                    start=(g == 0),
                    stop=(g == KT - 1),
                )
            nc.vector.tensor_copy(rb_sb[0:B, :], pr[:, :])

            for b in range(B):
                pb = pb_pool.tile([P, JC], FP)
                nc.tensor.matmul(
                    pb[:, :],
                    sel_sb[:, b * P : (b + 1) * P],
                    rb_sb[:, :],
                    start=True,
                    stop=True,
                )
                nc.vector.tensor_copy(bc_sb[:, b, :], pb[:, :])
                for sc in range(S // P):
                    nc.sync.dma_start(
                        out_d[b, sc * P : (sc + 1) * P, :], bc_sb[:, b, :]
                    )

    nc.compile()
    return nc


def build_nc_a():
    return build_nc_a_raw() if USE_RAW else build_nc_a_tile()


def build_nc_b():
    return build_nc_b_raw() if USE_RAW else build_nc_b_tile()


def make_in_maps_a(condition, Wv, bv):
    ct = np.ascontiguousarray(np.asarray(condition, dtype=np.float32).T)
    wvT = np.asarray(Wv, dtype=np.float32).T
    bv = np.asarray(bv, dtype=np.float32)
    id4 = np.eye(B, dtype=np.float32)
    in_maps = []
    for i in range(N_CORES):
        sl = slice(i * JC, (i + 1) * JC)
        in_maps.append(
            {
                "ct": ct,
                "wv_s": np.ascontiguousarray(wvT[:, sl]),
                "bv_s": np.ascontiguousarray(bv[sl].reshape(JC // P, P).T),
                "id4": id4,
            }
        )
    return in_maps


def make_in_maps_b(vt, Wo, bo):
    woT = np.asarray(Wo, dtype=np.float32).T
    bo = np.asarray(bo, dtype=np.float32)
    sel = np.zeros((B + 1, B * P), dtype=np.float32)
    for b in range(B):
        sel[b, b * P : (b + 1) * P] = 1.0
    sel[B, :] = 1.0
    in_maps = []
    for i in range(N_CORES):
        sl = slice(i * JC, (i + 1) * JC)
        in_maps.append(
            {
                "vt": vt,
                "wo_s": np.ascontiguousarray(woT[:, sl]),
                "bo_s": np.ascontiguousarray(bo[sl]).reshape(1, JC),
                "sel": sel,
            }
        )
    return in_maps


_NC_CACHE = None


def get_ncs():
    global _NC_CACHE
    if _NC_CACHE is None:
        _NC_CACHE = (build_nc_a(), build_nc_b())
    return _NC_CACHE


def kernel(**inputs):
    nc_a, nc_b = get_ncs()
    cores = list(range(N_CORES))

    res_a = run_bass_kernel_spmd(
        nc_a,
        make_in_maps_a(inputs["condition"], inputs["Wv"], inputs["bv"]),
        core_ids=cores,
    )
    vt = np.ascontiguousarray(
        np.concatenate([r["vt_s"] for r in res_a.results], axis=0)
    )

    res_b = run_bass_kernel_spmd(
        nc_b,
        make_in_maps_b(vt, inputs["Wo"], inputs["bo"]),
        core_ids=cores,
    )
    out = np.concatenate([r["out"] for r in res_b.results], axis=-1)
    return out


# revision 25
# speedup vs baseline: 1.2821x; 1.1230x over previous
"""Trainium2 Bass kernel for CrossAttentionConditionInjection.

Math note: in the reference, K and V are projections of a single per-batch
condition vector broadcast identically across all S key positions.  The
attention scores are therefore constant along the softmax axis, softmax is
exactly uniform (1/S each), and the attention output is the mean of S
identical V rows, i.e. V itself.  The whole module collapses exactly to

    out[b, s, :] = (condition[b] @ Wv.T + bv) @ Wo.T + bo      (for every s)

independent of hidden_states / Wq / bq / Wk / bk.  (S = 1024 is a power of
two, so even the fp32 softmax-average path is bit-exact against this.)

Device strategy (8 NeuronCores on one trn2 chip, SPMD, two small NEFFs —
a collective-based single NEFF was measured slower: any collective costs
~80us wall in this runtime, while a whole no-collective NEFF is ~12us):

  Launch A: Wv.T column-sharded 8x.  Core i computes
            vT[256i:256(i+1), :] = (condition @ Wv.T + bv).T[shard]
            and returns the (256, 4) shard.  Host concatenates to the
            full (2048, 4) vT (layout only).
  Launch B: Wo.T column-sharded 8x.  Core i computes
            r[:, shard] = vT.T @ Wo.T[:, shard], folds bo + the
            broadcast over sequence positions into one selector matmul
            per batch entry, and writes its (4, 1024, 256) output
            slice.  Host concatenates along channels (layout only).

Both launches are Tile kernels (USE_RAW=False): a raw-bass rewrite with
manual semaphores was measured slower (90us vs 82us) — Tile's per-chunk
DMA/compute pipelining beats its ~8us/NEFF barrier overhead here.
"""

import numpy as np

import concourse.bass as bass
import concourse.mybir as mybir
import concourse.tile as tile
from concourse import bacc
from concourse.bass_utils import run_bass_kernel_spmd
from concourse.masks import make_identity

B = 4
S = 1024
D = 2048
N_CORES = 8
JC = D // N_CORES  # 256 channels per core (v-shard in A, out-shard in B)
P = 128
KT = D // P  # 16 k-chunks
FP = mybir.dt.float32

USE_RAW = False

N_WARM = 8  # junk matmuls to lift the PE HAM clock gate while DMAs stream


def _new_nc():
    return bacc.Bacc(
        "TRN2",
        target_bir_lowering=False,
        debug=False,
        enable_asserts=False,
        num_devices=N_CORES,
    )


def build_nc_a_raw():
    nc = _new_nc()
    ct_d = nc.dram_tensor("ct", [D, B], FP, kind="ExternalInput").ap()
    wv_d = nc.dram_tensor("wv_s", [D, JC], FP, kind="ExternalInput").ap()
    bv_d = nc.dram_tensor("bv_s", [P, JC // P], FP, kind="ExternalInput").ap()
    id4_d = nc.dram_tensor("id4", [B, B], FP, kind="ExternalInput").ap()
    vt_d = nc.dram_tensor("vt_s", [JC, B], FP, kind="ExternalOutput").ap()

    N_IN = 3 + KT  # ct, bv, id4, wv x16

    with (
        nc.semaphore("s_in") as s_in,
        nc.semaphore("s_h0") as s_h0,
        nc.semaphore("s_h1") as s_h1,
        nc.semaphore("s_wu") as s_wu,
        nc.semaphore("s_pv") as s_pv,
        nc.semaphore("s_vl") as s_vl,
        nc.semaphore("s_mm") as s_mm,
        nc.semaphore("s_vt") as s_vt,
        nc.semaphore("s_out") as s_out,
        nc.sbuf_tensor("ct_sb", [P, KT * B], FP) as ct_sb,
        nc.sbuf_tensor("wv_sb", [P, KT * JC], FP) as wv_sb,
        nc.sbuf_tensor("bv_sb", [P, JC // P], FP) as bv_sb,
        nc.sbuf_tensor("vl_sb", [B, JC], FP) as vl_sb,
        nc.sbuf_tensor("vtl_sb", [P, (JC // P) * B], FP) as vtl_sb,
        nc.sbuf_tensor("id4_sb", [B, B], FP) as id4_sb,
        nc.sbuf_tensor("wup_sb", [P, P], FP) as wup_sb,
        nc.psum_tensor("pwu", [P, 512], FP) as pwu,
        nc.psum_tensor("pv", [B, 512], FP) as pv,
        nc.psum_tensor("pt0", [P, 512], FP) as pt0,
        nc.psum_tensor("pt1", [P, 512], FP) as pt1,
        nc.Block() as block,
    ):

        @block.sync
        def _(sync):
            sync.dma_start(id4_sb[:, :], id4_d[:, :]).then_inc(s_in, 16)
            sync.dma_start(
                ct_sb[:, :].rearrange("p (t b) -> p t b", t=KT),
                ct_d.rearrange("(t p) b -> p t b", p=P),
            ).then_inc(s_in, 16)
            sync.dma_start(bv_sb[:, :], bv_d[:, :]).then_inc(s_in, 16)
            for t in range(KT):
                sync.dma_start(
                    wv_sb[:, t * JC : (t + 1) * JC], wv_d[t * P : (t + 1) * P, :]
                ).then_inc(s_h0 if t < KT // 2 else s_h1, 16)
            sync.wait_ge(s_vt, 2)
            sync.dma_start(
                vt_d.rearrange("(g p) b -> p g b", p=P),
                vtl_sb[:, :].rearrange("p (g b) -> p g b", g=JC // P),
            ).then_inc(s_out, 16)
            sync.wait_ge(s_out, 16)

        @block.vector
        def _(vector):
            vector.memset(wup_sb[:, :], 0.0).then_inc(s_wu, 1)
            vector.wait_ge(s_pv, 1)
            vector.tensor_copy(vl_sb[:, :], pv[:, 0:JC]).then_inc(s_vl, 1)
            for g in range(JC // P):
                pt = pt0 if g == 0 else pt1
                vector.wait_ge(s_mm, g + 1)
                vector.tensor_scalar_add(
                    vtl_sb[:, g * B : (g + 1) * B], pt[:, 0:B], bv_sb[:, g : g + 1]
                ).then_inc(s_vt, 1)

        @block.tensor
        def _(tensor):
            tensor.wait_ge(s_wu, 1)
            for w in range(N_WARM):
                tensor.matmul(
                    pwu[:, 0:P], wup_sb[:, :], wup_sb[:, :], start=True, stop=True
                )
            tensor.wait_ge(s_in, 3 * 16)
            tensor.wait_ge(s_h0, (KT // 2) * 16)
            for t in range(KT):
                if t == KT // 2:
                    tensor.wait_ge(s_h1, (KT // 2) * 16)
                mm = tensor.matmul(
                    pv[:, 0:JC],
                    ct_sb[:, t * B : (t + 1) * B],
                    wv_sb[:, t * JC : (t + 1) * JC],
                    start=(t == 0),
                    stop=(t == KT - 1),
                )
            mm.then_inc(s_pv, 1)
            tensor.wait_ge(s_vl, 1)
            for g in range(JC // P):
                pt = pt0 if g == 0 else pt1
                tensor.transpose(
                    pt[:, 0:B], vl_sb[:, g * P : (g + 1) * P], id4_sb[:, :]
                ).then_inc(s_mm, 1)

    nc.compile()
    return nc


def build_nc_b_raw():
    nc = _new_nc()
    vt_d = nc.dram_tensor("vt", [D, B], FP, kind="ExternalInput").ap()
    wo_d = nc.dram_tensor("wo_s", [D, JC], FP, kind="ExternalInput").ap()
    bo_d = nc.dram_tensor("bo_s", [1, JC], FP, kind="ExternalInput").ap()
    sel_d = nc.dram_tensor("sel", [B + 1, B * P], FP, kind="ExternalInput").ap()
    out_d = nc.dram_tensor("out", [B, S, JC], FP, kind="ExternalOutput").ap()

    N_IN = 3 + KT  # vt, bo, sel, wo x16

    with (
        nc.semaphore("s_in") as s_in,
        nc.semaphore("s_h0") as s_h0,
        nc.semaphore("s_h1") as s_h1,
        nc.semaphore("s_wu") as s_wu,
        nc.semaphore("s_r") as s_r,
        nc.semaphore("s_rb") as s_rb,
        nc.semaphore("s_bct") as s_bct,
        nc.semaphore("s_bc") as s_bc,
        nc.semaphore("s_out") as s_out,
        nc.sbuf_tensor("vt_sb", [P, KT * B], FP) as vt_sb,
        nc.sbuf_tensor("wo_sb", [P, KT * JC], FP) as wo_sb,
        nc.sbuf_tensor("rb_sb", [B + 1, JC], FP) as rb_sb,
        nc.sbuf_tensor("sel_sb", [B + 1, B * P], FP) as sel_sb,
        nc.sbuf_tensor("bc_sb", [P, B * JC], FP) as bc_sb,
        nc.sbuf_tensor("wup_sb", [P, P], FP) as wup_sb,
        nc.psum_tensor("pwu", [P, 512], FP) as pwu,
        nc.psum_tensor("pr", [B, 512], FP) as pr,
        nc.psum_tensor("pb0", [P, 512], FP) as pb0,
        nc.psum_tensor("pb1", [P, 512], FP) as pb1,
        nc.Block() as block,
    ):

        @block.sync
        def _(sync):
            sync.dma_start(
                vt_sb[:, :].rearrange("p (g b) -> p g b", g=KT),
                vt_d.rearrange("(g p) b -> p g b", p=P),
            ).then_inc(s_in, 16)
            sync.dma_start(rb_sb[B : B + 1, :], bo_d[:, :]).then_inc(s_in, 16)
            sync.dma_start(sel_sb[:, :], sel_d[:, :]).then_inc(s_in, 16)
            for g in range(KT):
                sync.dma_start(
                    wo_sb[:, g * JC : (g + 1) * JC], wo_d[g * P : (g + 1) * P, :]
                ).then_inc(s_h0 if g < KT // 2 else s_h1, 16)
            for b in range(B):
                sync.wait_ge(s_bc, b + 1)
                for sc in range(S // P):
                    sync.dma_start(
                        out_d[b, sc * P : (sc + 1) * P, :],
                        bc_sb[:, b * JC : (b + 1) * JC],
                    ).then_inc(s_out, 16)
            sync.wait_ge(s_out, B * (S // P) * 16)

        @block.vector
        def _(vector):
            vector.memset(wup_sb[:, :], 0.0).then_inc(s_wu, 1)
            vector.wait_ge(s_r, 1)
            vector.tensor_copy(rb_sb[0:B, :], pr[:, 0:JC]).then_inc(s_rb, 1)
            for b in range(B):
                pb = pb0 if b % 2 == 0 else pb1
                vector.wait_ge(s_bct, b + 1)
                vector.tensor_copy(
                    bc_sb[:, b * JC : (b + 1) * JC], pb[:, 0:JC]
                ).then_inc(s_bc, 1)

        @block.tensor
        def _(tensor):
            tensor.wait_ge(s_wu, 1)
            for w in range(N_WARM):
                tensor.matmul(
                    pwu[:, 0:P], wup_sb[:, :], wup_sb[:, :], start=True, stop=True
                )
            tensor.wait_ge(s_in, 3 * 16)
            tensor.wait_ge(s_h0, (KT // 2) * 16)
            for g in range(KT):
                if g == KT // 2:
                    tensor.wait_ge(s_h1, (KT // 2) * 16)
                mm = tensor.matmul(
                    pr[:, 0:JC],
                    vt_sb[:, g * B : (g + 1) * B],
                    wo_sb[:, g * JC : (g + 1) * JC],
                    start=(g == 0),
                    stop=(g == KT - 1),
                )
            mm.then_inc(s_r, 1)
            tensor.wait_ge(s_rb, 1)
            for b in range(B):
                pb = pb0 if b % 2 == 0 else pb1
                if b >= 2:
                    tensor.wait_ge(s_bc, b - 1)
                tensor.matmul(
                    pb[:, 0:JC],
                    sel_sb[:, b * P : (b + 1) * P],
                    rb_sb[:, :],
                    start=True,
                    stop=True,
                ).then_inc(s_bct, 1)

    nc.compile()
    return nc


def build_nc_a_tile():
    nc = _new_nc()
    ct_d = nc.dram_tensor("ct", [D, B], FP, kind="ExternalInput").ap()
    wv_d = nc.dram_tensor("wv_s", [D, JC], FP, kind="ExternalInput").ap()
    bv_d = nc.dram_tensor("bv_s", [P, JC // P], FP, kind="ExternalInput").ap()
    id4_d = nc.dram_tensor("id4", [B, B], FP, kind="ExternalInput").ap()
    vt_d = nc.dram_tensor("vt_s", [JC, B], FP, kind="ExternalOutput").ap()

    with tile.TileContext(nc) as tc:
        with (
            tc.tile_pool(name="work", bufs=1) as work,
            tc.tile_pool(name="pv", bufs=1, space="PSUM") as pv_pool,
            tc.tile_pool(name="pt", bufs=2, space="PSUM") as pt_pool,
        ):
            wv_sb = work.tile([P, KT, JC], FP)
            ct_sb = work.tile([P, KT, B], FP)
            bv_sb = work.tile([P, JC // P], FP)
            vl_sb = work.tile([B, JC], FP)
            vtl_sb = work.tile([P, JC // P, B], FP)
            id4_sb = work.tile([B, B], FP)
            nc.sync.dma_start(id4_sb[:, :], id4_d[:, :])

            nc.sync.dma_start(ct_sb[:, :, :], ct_d.rearrange("(t p) b -> p t b", p=P))
            for t in range(KT):
                nc.sync.dma_start(wv_sb[:, t, :], wv_d[t * P : (t + 1) * P, :])
            nc.sync.dma_start(bv_sb[:, :], bv_d[:, :])

            pv = pv_pool.tile([B, JC], FP)
            for t in range(KT):
                nc.tensor.matmul(
                    pv[:, :],
                    ct_sb[:, t, :],
                    wv_sb[:, t, :],
                    start=(t == 0),
                    stop=(t == KT - 1),
                )
            nc.vector.tensor_copy(vl_sb[:, :], pv[:, :])

            for g in range(JC // P):
                pt = pt_pool.tile([P, B], FP)
                nc.tensor.transpose(
                    pt[:, :], vl_sb[:, g * P : (g + 1) * P], id4_sb[:, :]
                )
                nc.vector.tensor_scalar_add(
                    vtl_sb[:, g, :], pt[:, :], bv_sb[:, g : g + 1]
                )
            nc.sync.dma_start(
                vt_d.rearrange("(g p) b -> p g b", p=P), vtl_sb[:, :, :]
            )

    nc.compile()
    return nc


def build_nc_b_tile():
    nc = _new_nc()
    vt_d = nc.dram_tensor("vt", [D, B], FP, kind="ExternalInput").ap()
    wo_d = nc.dram_tensor("wo_s", [D, JC], FP, kind="ExternalInput").ap()
    bo_d = nc.dram_tensor("bo_s", [1, JC], FP, kind="ExternalInput").ap()
    sel_d = nc.dram_tensor("sel", [B + 1, B * P], FP, kind="ExternalInput").ap()
    out_d = nc.dram_tensor("out", [B, S, JC], FP, kind="ExternalOutput").ap()

    with tile.TileContext(nc) as tc:
        with (
            tc.tile_pool(name="work", bufs=1) as work,
            tc.tile_pool(name="pr", bufs=1, space="PSUM") as pr_pool,
            tc.tile_pool(name="pb", bufs=2, space="PSUM") as pb_pool,
        ):
            wo_sb = work.tile([P, KT, JC], FP)
            vt_sb = work.tile([P, KT, B], FP)
            rb_sb = work.tile([B + 1, JC], FP)
            sel_sb = work.tile([B + 1, B * P], FP)
            bc_sb = work.tile([P, B, JC], FP)

            nc.sync.dma_start(vt_sb[:, :, :], vt_d.rearrange("(g p) b -> p g b", p=P))
            for g in range(KT):
                nc.sync.dma_start(wo_sb[:, g, :], wo_d[g * P : (g + 1) * P, :])
            nc.sync.dma_start(rb_sb[B : B + 1, :], bo_d[:, :])
            nc.sync.dma_start(sel_sb[:, :], sel_d[:, :])

            pr = pr_pool.tile([B, JC], FP)
            for g in range(KT):
                nc.tensor.matmul(
                    pr[:, :],
                    vt_sb[:, g, :],
                    wo_sb[:, g, :],
                    start=(g == 0),
                    stop=(g == KT - 1),
                )
            nc.vector.tensor_copy(rb_sb[0:B, :], pr[:, :])

            for b in range(B):
                pb = pb_pool.tile([P, JC], FP)
                nc.tensor.matmul(
                    pb[:, :],
                    sel_sb[:, b * P : (b + 1) * P],
                    rb_sb[:, :],
                    start=True,
                    stop=True,
                )
                nc.vector.tensor_copy(bc_sb[:, b, :], pb[:, :])
                for sc in range(S // P):
                    nc.sync.dma_start(
                        out_d[b, sc * P : (sc + 1) * P, :], bc_sb[:, b, :]
                    )

    nc.compile()
    return nc


def build_nc_a():
    return build_nc_a_raw() if USE_RAW else build_nc_a_tile()


def build_nc_b():
    return build_nc_b_raw() if USE_RAW else build_nc_b_tile()


def make_in_maps_a(condition, Wv, bv):
    ct = np.ascontiguousarray(np.asarray(condition, dtype=np.float32).T)
    wvT = np.asarray(Wv, dtype=np.float32).T
    bv = np.asarray(bv, dtype=np.float32)
    id4 = np.eye(B, dtype=np.float32)
    in_maps = []
    for i in range(N_CORES):
        sl = slice(i * JC, (i + 1) * JC)
        in_maps.append(
            {
                "ct": ct,
                "wv_s": np.ascontiguousarray(wvT[:, sl]),
                "bv_s": np.ascontiguousarray(bv[sl].reshape(JC // P, P).T),
                "id4": id4,
            }
        )
    return in_maps


def make_in_maps_b(vt, Wo, bo):
    woT = np.asarray(Wo, dtype=np.float32).T
    bo = np.asarray(bo, dtype=np.float32)
    sel = np.zeros((B + 1, B * P), dtype=np.float32)
    for b in range(B):
        sel[b, b * P : (b + 1) * P] = 1.0
    sel[B, :] = 1.0
    in_maps = []
    for i in range(N_CORES):
        sl = slice(i * JC, (i + 1) * JC)
        in_maps.append(
            {
                "vt": vt,
                "wo_s": np.ascontiguousarray(woT[:, sl]),
                "bo_s": np.ascontiguousarray(bo[sl]).reshape(1, JC),
                "sel": sel,
            }
        )
    return in_maps


_NC_CACHE = None


def get_ncs():
    global _NC_CACHE
    if _NC_CACHE is None:
        _NC_CACHE = (build_nc_a(), build_nc_b())
    return _NC_CACHE


def kernel(**inputs):
    nc_a, nc_b = get_ncs()
    cores = list(range(N_CORES))

    res_a = run_bass_kernel_spmd(
        nc_a,
        make_in_maps_a(inputs["condition"], inputs["Wv"], inputs["bv"]),
        core_ids=cores,
    )
    vt = np.ascontiguousarray(
        np.concatenate([r["vt_s"] for r in res_a.results], axis=0)
    )

    res_b = run_bass_kernel_spmd(
        nc_b,
        make_in_maps_b(vt, inputs["Wo"], inputs["bo"]),
        core_ids=cores,
    )
    out = np.concatenate([r["out"] for r in res_b.results], axis=-1)
    return out
